# revision 45
# baseline (speedup 1.0000x reference)
"""EntityEncoder (gnn_message_passing) Trainium2 kernel — 8-core SPMD.

Strategy: edges are pre-partitioned on the host into 8 contiguous,
entity-aligned, edge-balanced shards (entity_indices is sorted, so each
entity's edges land wholly on one core — no cross-core collectives).
Within a core, segments are LPT-packed into 10 blocks of <=128 segments /
<=1280 edges.  All HBM traffic is bf16.  The host folds the prompt-score,
count-score and scorer bias into a per-edge prescore, gathers per-edge
count embeddings (with an appended ones column that yields the softmax
denominator for free), and pre-tiles the projection weights.  On device:
one fused 2304-col dot per 128-edge chunk (vector), exp on scalar,
one-hot segment matmuls on tensor for the three segment reductions,
PE transposes of the [seg,feat] aggregates, then bf16 output projections.
Projection bias and the final row scatter are applied on the host.
"""
import sys
import numpy as np
import ml_dtypes

for _p in ("/root/.axon_site", "/root/.axon_site/_ro/trn_rl_repo",
           "/root/.axon_site/_ro/pypackages"):
    if _p not in sys.path:
        sys.path.append(_p)

import bass_rust
import concourse.bass as bass
import concourse.mybir as mybir
import concourse.tile as tile
from concourse.vector_clock import ScopedClock
from contextlib import ExitStack

BF16 = ml_dtypes.bfloat16
dt = mybir.dt
Alu = mybir.AluOpType
Act = mybir.ActivationFunctionType

# problem shape (hardcoded per contest contract)
N_CORES = 8
N = 100_000
P = 64
E = 10_000
D = 768
C = 1000
OUT = 5120
# per-core packing
NBLK = 10
SPB = 128                # segs per block
CH = 10                  # chunks (of 128 edges) per block
EPB = CH * 128           # edges per block = 1280
NL = NBLK * EPB          # 12800 edge slots per core
E_PAD = NBLK * SPB       # 1280 seg slots per core
OH = OUT // 5            # 1024-wide output slab
PAD_SEG = 999.0
EDA = 1024               # pass-A score cols: ent0(512) + rel0(512)
EDB = 512                # pass-B score cols: ent1(256) + rel1(256)
CWB = 264                # pass-B count cols: ce[511:768] + 7 pad
BGA = 1536               # pass-A tensor: e0 | r0 | [ones|ce0:511]
TLB = EDB + CWB          # pass-B tensor: e1 | r1 | ceB = 776


class _TileContextSplitDrain(tile.TileContext):
    """This container's walrus accepts only ONE sync wait per instruction
    ("Too many sync wait commands" in setupSyncWait). Split every extra wait
    onto a standalone same-engine NoOp placed immediately before the
    instruction — identical semantics, one wait per instruction."""

    def _lower_ordered_insts(self, ordered):
        for insts in ordered.values():
            if not any(
                i.sync_info is not None and len(i.sync_info.on_wait) > 1
                for i in insts
            ):
                continue
            new = []
            for inst in insts:
                si = inst.sync_info
                if si is not None and len(si.on_wait) > 1:
                    waits = list(si.on_wait)
                    for w in waits[:-1]:
                        nop = bass_rust.InstNoOp(
                            name=self.nc.get_next_instruction_name(),
                            ins=[], outs=[])
                        nop.engine = inst.engine
                        nop.sync_info = bass_rust.SyncInfo(
                            on_wait=[w], on_update=[])
                        new.append(nop)
                    si.on_wait = waits[-1:]
                new.append(inst)
            insts[:] = new
        return super()._lower_ordered_insts(ordered)

    def _drain_and_barrier(self, tick_clock, wait_clock):
        nc = self.nc
        drain_inst = nc.sync.drain()
        wait_clock.add_sem_waits(
            drain_inst.ins, ScopedClock({None: tick_clock.global_clock})
        )
        si = drain_inst.ins.sync_info
        if si is not None and len(si.on_wait) > 1:
            waits = list(si.on_wait)
            si.on_wait = waits[:1]
            for w in waits[1:]:
                n = nc.sync.nop()
                n.ins.sync_info = bass_rust.SyncInfo(on_wait=[w], on_update=[])
        nc.all_engine_barrier()
        assert self.sems is not None
        popped = nc._tile_sem_poison_stack.pop()
        assert popped is self._sem_poison
        nc.clear_and_free_semaphores(list(self.sems.allocated().values()))
        nc.all_engine_barrier()


# --------------------------------------------------------------------------
# host-side sharding / packing
# --------------------------------------------------------------------------

def _shard_and_pack(entity_indices):
    Nn = entity_indices.shape[0]
    starts = np.searchsorted(entity_indices, np.arange(E + 1))
    ideal = (np.arange(1, N_CORES) * Nn) // N_CORES
    ent_bnd = [0]
    for t in ideal:
        s = int(np.searchsorted(starts, t))
        if s > 0 and abs(int(starts[s - 1]) - int(t)) < abs(int(starts[s]) - int(t)):
            s -= 1
        ent_bnd.append(s)
    ent_bnd.append(E)

    cores = []
    for c in range(N_CORES):
        e_lo, e_hi = ent_bnd[c], ent_bnd[c + 1]
        segs = np.arange(e_lo, e_hi)
        sizes = (starts[e_lo + 1 : e_hi + 1] - starts[e_lo:e_hi]).astype(np.int64)
        n_edges = int(sizes.sum())
        assert e_hi - e_lo <= E_PAD and n_edges <= NL
        order = np.argsort(-sizes, kind="stable")
        blk_edges = [0] * NBLK
        blk_nseg = [0] * NBLK
        blk_segs = [[] for _ in range(NBLK)]
        for idx in order:
            sz = int(sizes[idx])
            best = -1
            for b in sorted(range(NBLK), key=lambda b: blk_edges[b]):
                if blk_nseg[b] < SPB and blk_edges[b] + sz <= EPB:
                    best = b
                    break
            assert best >= 0, "block packing overflow"
            blk_segs[best].append(int(segs[idx]))
            blk_edges[best] += sz
            blk_nseg[best] += 1
        perm = np.full(NL, -1, dtype=np.int64)
        seg_local = np.full(NL, PAD_SEG, dtype=np.float32)
        row2seg = np.full(E_PAD, -1, dtype=np.int64)
        inv_cnt = np.zeros(E_PAD, dtype=np.float32)
        for b in range(NBLK):
            pos = b * EPB
            for j, s in enumerate(blk_segs[b]):
                row = b * SPB + j
                row2seg[row] = s
                n = int(starts[s + 1] - starts[s])
                if n > 0:
                    inv_cnt[row] = 1.0 / n
                perm[pos : pos + n] = np.arange(starts[s], starts[s + 1])
                seg_local[pos : pos + n] = float(j)
                pos += n
        cores.append(dict(perm=perm, seg_local=seg_local, row2seg=row2seg,
                          inv_cnt=inv_cnt))
    return cores


# --------------------------------------------------------------------------
# device kernel
# --------------------------------------------------------------------------

def _build_nc():
    nc = bass.Bass("TRN2", target_bir_lowering=False, debug=False,
                   num_devices=N_CORES)

    f32, bf, i32 = dt.float32, dt.bfloat16, dt.int32
    # pass-A edge features (freed mid-block): [e0 512 | r0 512 | ones+ce0 512]
    biga_d = nc.dram_tensor("biga", [NL, BGA], bf, kind="ExternalInput")
    # pass-B tails (small, deep-buffered): [e1 256 | r1 256 | ceB 264]
    tailb_d = nc.dram_tensor("tailb", [NL, TLB], bf, kind="ExternalInput")
    sp_d = nc.dram_tensor("sp", [NL, 2], f32, kind="ExternalInput")
    icnt_d = nc.dram_tensor("inv_cnt", [E_PAD], f32, kind="ExternalInput")
    wenra_d = nc.dram_tensor("wenra", [128, EDA], bf, kind="ExternalInput")
    wenrb_d = nc.dram_tensor("wenrb", [128, EDB], bf, kind="ExternalInput")
    wtr_d = nc.dram_tensor("wtr", [5 * 128, 12 * OH], bf, kind="ExternalInput")
    wte_d = nc.dram_tensor("wte", [5 * 128, 6 * OH], bf, kind="ExternalInput")
    orel_d = nc.dram_tensor("orel", [5 * NBLK * 128, OH], bf,
                            kind="ExternalOutput")
    oent_d = nc.dram_tensor("oent", [5 * NBLK * 128, OH], bf,
                            kind="ExternalOutput")

    with _TileContextSplitDrain(nc) as tc, ExitStack() as es:
        const = es.enter_context(tc.tile_pool(name="const", bufs=1))
        accp = es.enter_context(tc.tile_pool(name="accp", bufs=1))

        # ---- constants ----
        iota_seg = const.tile([128, 128], bf)
        ident = const.tile([128, 128], bf)
        with tc.tile_pool(name="setup", bufs=1) as setup:
            iota_i = setup.tile([128, 128], i32)
            nc.gpsimd.iota(iota_i[:], pattern=[[1, 128]], base=0,
                           channel_multiplier=0)
            nc.vector.tensor_copy(iota_seg[:], iota_i[:])
            iota_ci = setup.tile([128, 1], i32)
            nc.gpsimd.iota(iota_ci[:], pattern=[[0, 1]], base=0,
                           channel_multiplier=1)
            iota_col = setup.tile([128, 1], f32)
            nc.vector.tensor_copy(iota_col[:], iota_ci[:])
            nc.vector.tensor_scalar(out=ident[:], in0=iota_seg[:],
                                    scalar1=iota_col[:],
                                    scalar2=None, op0=Alu.is_equal)

        wenra = const.tile([128, EDA], bf)
        nc.sync.dma_start(wenra[:], wenra_d.ap())
        wenrb = const.tile([128, EDB], bf)
        nc.sync.dma_start(wenrb[:], wenrb_d.ap())
        icnt_sb = const.tile([128, NBLK], f32)
        nc.sync.dma_start(
            icnt_sb[:], icnt_d.ap().rearrange("(b p) -> p b", p=128)
        )
        invd_sb = accp.tile([128, NBLK], f32)

        # resident transposed aggregates, one tile per (feat-chunk, block):
        # t 0-5 = relation, 6-11 = count emb  -> relcat (K=12 chunks)
        # t 0-5 of entT = entity              -> ent (K=6 chunks)
        relcatT = [[accp.tile([128, 128], bf, name=f"relcatT{t}_{b}",
                              tag=f"relcatT{t}_{b}") for b in range(NBLK)]
                   for t in range(12)]
        entT = [[accp.tile([128, 128], bf, name=f"entT{t}_{b}",
                           tag=f"entT{t}_{b}") for b in range(NBLK)]
                for t in range(6)]

        # ---- merged aggregation + projection (Tile interleaves by deps) ----
        HE = CH // 2  # 5 edges per partition per half-block
        with tc.tile_pool(name="edges", bufs=2) as edges, \
             tc.tile_pool(name="chunkp", bufs=2) as chunkp, \
             tc.tile_pool(name="evac", bufs=2) as evac, \
             tc.tile_pool(name="wpool", bufs=2) as wpool, \
             tc.tile_pool(name="outp", bufs=2) as outp, \
             tc.tile_pool(name="psagg", bufs=1, space="PSUM") as psagg, \
             tc.tile_pool(name="pp", bufs=2, space="PSUM") as pp:
            def emit_transposes(items):
                # items: list of (src_slice, dst_tile); alternate evac engine
                for i, (src, dst) in enumerate(items):
                    pt = pp.tile([128, 128], bf, tag="pt")
                    nc.tensor.transpose(pt[:], src, ident[:])
                    if i % 2 == 0:
                        nc.scalar.activation(dst[:], pt[:], Act.Copy)
                    else:
                        nc.vector.tensor_copy(dst[:], pt[:])

            def emit_pso(Tt, wt, KC, sblk, oc5, stage, scalar_evac=False):
                pso = pp.tile([128, 512], f32, tag="pso", bufs=3)
                for k in range(KC):
                    nc.tensor.matmul(
                        pso[:], Tt[k][sblk][:],
                        wt[:, k * OH + oc5 * 512 : k * OH + oc5 * 512 + 512],
                        start=(k == 0), stop=(k == KC - 1))
                if oc5 % 2 == 0 and not scalar_evac:
                    nc.vector.tensor_copy(
                        stage[:, oc5 * 512 : (oc5 + 1) * 512], pso[:])
                else:
                    nc.scalar.activation(
                        stage[:, oc5 * 512 : (oc5 + 1) * 512], pso[:],
                        Act.Copy)

            def emit_out_dma(o_d, h, sblk, stage):
                nc.scalar.dma_start(
                    o_d.ap()[(h * NBLK + sblk) * 128 :
                             (h * NBLK + sblk + 1) * 128, :],
                    stage[:],
                )

            wt_ent_h0 = None
            pend_tr = []   # deferred transposes of the previous block
            for b in range(NBLK):
                halves = []
                for hb in range(2):
                    r0 = b * EPB + hb * (EPB // 2)
                    r1 = r0 + EPB // 2
                    biga = edges.tile([128, HE * BGA], bf, tag="biga")
                    nc.sync.dma_start(
                        biga[:],
                        biga_d.ap()[r0:r1, :].rearrange("(p j) d -> p j d", j=HE),
                    )
                    tailb = edges.tile([128, HE * TLB], bf, tag="tailb", bufs=3)
                    nc.sync.dma_start(
                        tailb[:],
                        tailb_d.ap()[r0:r1, :].rearrange("(p j) d -> p j d", j=HE),
                    )
                    sph = edges.tile([128, HE * 2], f32, tag="sph", bufs=3)
                    nc.sync.dma_start(
                        sph[:],
                        sp_d.ap()[r0:r1, :].rearrange("(p j) c -> p j c", j=HE),
                    )
                    halves.append((biga, tailb, sph))
                    if b == 0 and hb == 1:
                        # gate the (gpsimd-queued) weight stream behind the
                        # LAST block-0 edge load so startup DMA bandwidth
                        # goes entirely to block 0
                        gate = chunkp.tile([1, 1], bf, tag="gate", bufs=1)
                        nc.gpsimd.tensor_copy(gate[:], tailb[0:1, 0:1])

                if b == 1:
                    # ent-h0 weight slab on a DEDICATED tag (bufs=1) so its
                    # long lifetime cannot block the main "wt" ring; queued
                    # on gpsimd behind the block-0 gate
                    wt_ent_h0 = wpool.tile([128, 6 * OH], bf, tag="wt0",
                                           bufs=1)
                    nc.gpsimd.dma_start(wt_ent_h0[:], wte_d.ap()[0:128, :])
                estage = None
                if b >= 2:
                    # one ready ent-h0 projection iteration per block: its two
                    # 6-MM pso groups fill the pass-transition and block-
                    # boundary stalls on the tensor queue
                    es_blk = b - 2
                    estage = outp.tile([128, OH], bf, tag="stage", bufs=4)

                # score + one-hot build for all 10 chunks (persist across
                # the two aggregation passes)
                ohs, ohxs = [], []
                for j in range(CH):
                    biga, tailb, sph = halves[j // HE]
                    jj = j % HE
                    slc = sph[:, 2 * jj : 2 * jj + 1]
                    prc = sph[:, 2 * jj + 1 : 2 * jj + 2]
                    scra = chunkp.tile([128, EDA], bf, tag="scra")
                    saA = chunkp.tile([128, 1], f32, tag=f"saA{j}")
                    nc.vector.scalar_tensor_tensor(
                        out=scra[:], in0=biga[:, jj * BGA : jj * BGA + EDA],
                        scalar=1.0, in1=wenra[:],
                        op0=Alu.mult, op1=Alu.mult, accum_out=saA[:])
                    scrb = chunkp.tile([128, EDB], bf, tag="scrb")
                    saB = chunkp.tile([128, 1], f32, tag=f"saB{j}")
                    nc.vector.scalar_tensor_tensor(
                        out=scrb[:], in0=tailb[:, jj * TLB : jj * TLB + EDB],
                        scalar=1.0, in1=wenrb[:],
                        op0=Alu.mult, op1=Alu.mult, accum_out=saB[:])
                    sa = chunkp.tile([128, 1], f32, tag=f"sa{j}")
                    nc.vector.tensor_scalar(out=sa[:], in0=saA[:],
                                            scalar1=saB[:], scalar2=None,
                                            op0=Alu.add)
                    ex_ = chunkp.tile([128, 1], f32, tag=f"ex{j}")
                    nc.scalar.activation(ex_[:], sa[:], Act.Exp, bias=prc)
                    oh = chunkp.tile([128, 128], bf, tag=f"oh{j}")
                    nc.vector.tensor_scalar(out=oh[:], in0=iota_seg[:],
                                            scalar1=slc,
                                            scalar2=None, op0=Alu.is_equal)
                    ohx = chunkp.tile([128, 128], bf, tag=f"ohx{j}")
                    nc.vector.tensor_scalar(out=ohx[:], in0=iota_seg[:],
                                            scalar1=slc,
                                            scalar2=ex_[:],
                                            op0=Alu.is_equal, op1=Alu.mult)
                    ohs.append(oh)
                    ohxs.append(ohx)

                relsb = evac.tile([128, D], bf, tag="relsb")
                cntsb = evac.tile([128, D], bf, tag="cntsb")
                entsb = evac.tile([128, D], bf, tag="entsb")

                # pass A: feature cols 0:512 (cntA col 0 is the ones column,
                # so psA_cnt[:,0] accumulates the softmax denominator)
                psA_rel = psagg.tile([128, 512], f32, tag="ps_rel")
                psA_cnt = psagg.tile([128, 512], f32, tag="ps_cnt")
                psA_ent = psagg.tile([128, 512], f32, tag="ps_ent")
                for j in range(CH):
                    biga, tailb, sph = halves[j // HE]
                    jj = j % HE
                    ejA = biga[:, jj * BGA : jj * BGA + 512]
                    rjA = biga[:, jj * BGA + 512 : jj * BGA + 1024]
                    cjA = biga[:, jj * BGA + 1024 : jj * BGA + 1536]
                    st, sp_ = (j == 0), (j == CH - 1)
                    nc.tensor.matmul(psA_rel[:], ohxs[j][:], rjA,
                                     start=st, stop=sp_)
                    nc.tensor.matmul(psA_cnt[:], ohxs[j][:], cjA,
                                     start=st, stop=sp_)
                    nc.tensor.matmul(psA_ent[:], ohs[j][:], ejA,
                                     start=st, stop=sp_)

                # first half of the previous block's transposes fills the
                # pass-A -> pass-B evac latency on the tensor queue
                emit_transposes(pend_tr[:9])
                if estage is not None:
                    emit_pso(entT, wt_ent_h0, 6, es_blk, 0, estage,
                             scalar_evac=True)

                dmx = chunkp.tile([128, 1], f32, tag="dmx")
                nc.vector.tensor_scalar(out=dmx[:], in0=psA_cnt[:, 0:1],
                                        scalar1=1e-30, scalar2=None, op0=Alu.max)
                nc.vector.reciprocal(invd_sb[:, b : b + 1], dmx[:])
                nc.scalar.activation(relsb[:, 0:512], psA_rel[:], Act.Copy,
                                     scale=invd_sb[:, b : b + 1])
                nc.scalar.activation(cntsb[:, 0:511], psA_cnt[:, 1:512],
                                     Act.Copy, scale=invd_sb[:, b : b + 1])
                nc.scalar.activation(entsb[:, 0:512], psA_ent[:], Act.Copy,
                                     scale=icnt_sb[:, b : b + 1])

                # pass B: feature cols 512:768 (+ count tail)
                psB_rel = psagg.tile([128, 512], f32, tag="ps_rel")
                psB_cnt = psagg.tile([128, 512], f32, tag="ps_cnt")
                psB_ent = psagg.tile([128, 512], f32, tag="ps_ent")
                for j in range(CH):
                    biga, tailb, sph = halves[j // HE]
                    jj = j % HE
                    ejB = tailb[:, jj * TLB : jj * TLB + 256]
                    rjB = tailb[:, jj * TLB + 256 : jj * TLB + 512]
                    cjB = tailb[:, jj * TLB + 512 : (jj + 1) * TLB]
                    st, sp_ = (j == 0), (j == CH - 1)
                    nc.tensor.matmul(psB_rel[:, 0:256], ohxs[j][:], rjB,
                                     start=st, stop=sp_)
                    nc.tensor.matmul(psB_cnt[:, 0:CWB], ohxs[j][:], cjB,
                                     start=st, stop=sp_)
                    nc.tensor.matmul(psB_ent[:, 0:256], ohs[j][:], ejB,
                                     start=st, stop=sp_)

                # second half of the previous block's transposes fills the
                # block-boundary evac latency
                emit_transposes(pend_tr[9:])
                if estage is not None:
                    emit_pso(entT, wt_ent_h0, 6, es_blk, 1, estage,
                             scalar_evac=True)
                    emit_out_dma(oent_d, 0, es_blk, estage)

                nc.scalar.activation(relsb[:, 512:D], psB_rel[:, 0:256],
                                     Act.Copy, scale=invd_sb[:, b : b + 1])
                nc.scalar.activation(cntsb[:, 511:D], psB_cnt[:, 0:257],
                                     Act.Copy, scale=invd_sb[:, b : b + 1])
                nc.scalar.activation(entsb[:, 512:D], psB_ent[:, 0:256],
                                     Act.Copy, scale=icnt_sb[:, b : b + 1])

                pend_tr = []
                for t in range(6):
                    pend_tr.append((relsb[:, t * 128 : (t + 1) * 128],
                                    relcatT[t][b]))
                    pend_tr.append((cntsb[:, t * 128 : (t + 1) * 128],
                                    relcatT[6 + t][b]))
                    pend_tr.append((entsb[:, t * 128 : (t + 1) * 128],
                                    entT[t][b]))
            emit_transposes(pend_tr)

            # ---- projections (interleave with later aggregation blocks) ----
            for (Tt, wt_d, o_d, KC) in (
                (relcatT, wtr_d, orel_d, 12),
                (entT, wte_d, oent_d, 6),
            ):
                for h in range(5):
                    if Tt is entT and h == 0:
                        wt = wt_ent_h0          # hoisted; sblk 0..7 done
                        sblks = range(NBLK - 2, NBLK)
                    else:
                        wt = wpool.tile([128, KC * OH], bf, tag="wt")
                        nc.gpsimd.dma_start(
                            wt[:], wt_d.ap()[h * 128 : (h + 1) * 128, :])
                        sblks = range(NBLK)
                    for sblk in sblks:
                        stage = outp.tile([128, OH], bf, tag="stage", bufs=4)
                        emit_pso(Tt, wt, KC, sblk, 0, stage)
                        emit_pso(Tt, wt, KC, sblk, 1, stage)
                        emit_out_dma(o_d, h, sblk, stage)
    return nc


_NC_CACHE = None


def _get_nc():
    global _NC_CACHE
    if _NC_CACHE is None:
        _NC_CACHE = _build_nc()
    return _NC_CACHE


# --------------------------------------------------------------------------
# entry point
# --------------------------------------------------------------------------

def kernel(prompt_embs, entity_embs, neighbor_embs, relation_embs,
           count_table, scorer_W, scorer_b, rel_W, rel_b, ent_W, ent_b,
           counts, prompt_indices, entity_indices):
    from concourse.bass_utils import run_bass_kernel_spmd

    prompt_embs = np.asarray(prompt_embs, dtype=np.float32)
    entity_embs = np.asarray(entity_embs, dtype=np.float32)
    neighbor_embs = np.asarray(neighbor_embs, dtype=np.float32)
    relation_embs = np.asarray(relation_embs, dtype=np.float32)
    count_table = np.asarray(count_table, dtype=np.float32)
    scorer_W = np.asarray(scorer_W, dtype=np.float32)
    scorer_b = np.asarray(scorer_b, dtype=np.float32)
    rel_W = np.asarray(rel_W, dtype=np.float32)
    rel_b = np.asarray(rel_b, dtype=np.float32)
    ent_W = np.asarray(ent_W, dtype=np.float32)
    ent_b = np.asarray(ent_b, dtype=np.float32)
    counts = np.asarray(counts)
    prompt_indices = np.asarray(prompt_indices)
    entity_indices = np.asarray(entity_indices)

    cores = _shard_and_pack(entity_indices)

    # replicated (weight-derived) host prep
    w = scorer_W[0]
    pscore = (prompt_embs * w[None, :D]).sum(1) + scorer_b[0]     # fold bias
    cscore = (count_table * w[None, 4 * D :]).sum(1)
    w_ent, w_nbr, w_rel = w[D : 2 * D], w[2 * D : 3 * D], w[3 * D : 4 * D]
    nscore = (neighbor_embs * w_nbr[None, :]).sum(1)   # fold neighbor dot
    wenra = np.broadcast_to(
        np.concatenate([w_ent[0:512], w_rel[0:512]]).astype(BF16),
        (128, EDA)).copy()
    wenrb = np.broadcast_to(
        np.concatenate([w_ent[512:D], w_rel[512:D]]).astype(BF16),
        (128, EDB)).copy()

    # pass-A edge features [ent0:512 | rel0:512 | ones+ce], pass-B tails
    enra_full = np.empty((N, EDA), BF16)
    enra_full[:, 0:512] = entity_embs[:, 0:512].astype(BF16)
    enra_full[:, 512:] = relation_embs[:, 0:512].astype(BF16)
    enrb_full = np.empty((N, EDB), BF16)
    enrb_full[:, 0:256] = entity_embs[:, 512:D].astype(BF16)
    enrb_full[:, 256:512] = relation_embs[:, 512:D].astype(BF16)
    ct_bf = count_table.astype(BF16)
    # per-edge prescore = prompt + count + neighbor scores (+ bias)
    pres_full = (pscore[prompt_indices] + cscore[counts]
                 + nscore).astype(np.float32)

    # pre-tiled projection weights: [h*128+p, k*OH+c] = W[h*OH+c, k*128+p]
    def tile_w(W, KC):
        WT = np.ascontiguousarray(W.T).astype(BF16)          # [K*128, OUT]
        return np.ascontiguousarray(
            WT.reshape(KC, 128, 5, OH).transpose(2, 1, 0, 3)
        ).reshape(5 * 128, KC * OH)

    wtr = tile_w(rel_W, 12)
    wte = tile_w(ent_W, 6)

    in_maps = []
    for core in cores:
        perm = core["perm"]
        valid = perm >= 0
        src = np.where(valid, perm, 0)

        ce = ct_bf[counts[src]]
        ce[~valid] = 0
        biga = np.zeros((NL, BGA), BF16)
        biga[:, 0:EDA] = enra_full[src]
        biga[~valid, 0:EDA] = 0
        biga[:, 1024] = 1.0      # ones col -> softmax denominator (pass A)
        biga[~valid, 1024] = 0
        biga[:, 1025:1536] = ce[:, 0:511]
        tailb = np.zeros((NL, TLB), BF16)
        tailb[:, 0:512] = enrb_full[src]
        tailb[~valid, 0:512] = 0
        tailb[:, 512 : 512 + 257] = ce[:, 511:D]
        sp = np.zeros((NL, 2), np.float32)
        sp[:, 0] = core["seg_local"]
        sp[:, 1] = pres_full[src]
        sp[~valid, 1] = 0.0

        in_maps.append(dict(
            biga=biga, tailb=tailb,
            sp=sp, inv_cnt=core["inv_cnt"], wenra=wenra, wenrb=wenrb,
            wtr=wtr, wte=wte,
        ))

    nc = _get_nc()
    res = run_bass_kernel_spmd(nc, in_maps, list(range(N_CORES)))

    rel_out = np.zeros((E, OUT), np.float32)
    ent_out = np.zeros((E, OUT), np.float32)
    for c, core in enumerate(cores):
        rows = core["row2seg"]
        mask = rows >= 0
        # output DRAM layout [5h x 10blk x 128p, 1024c] -> [1280, 5120]
        orel = np.asarray(res.results[c]["orel"], dtype=np.float32)
        oent = np.asarray(res.results[c]["oent"], dtype=np.float32)
        orel = orel.reshape(5, NBLK * 128, OH).transpose(1, 0, 2).reshape(E_PAD, OUT)
        oent = oent.reshape(5, NBLK * 128, OH).transpose(1, 0, 2).reshape(E_PAD, OUT)
        rel_out[rows[mask]] = orel[mask]
        ent_out[rows[mask]] = oent[mask]
    rel_out += rel_b[None, :]
    ent_out += ent_b[None, :]
    return rel_out, ent_out
